# revision 24
# baseline (speedup 1.0000x reference)
"""Trainium2 Bass kernel for nn_Decomposeable (decomposable attention model).

Strategy: data-parallel over batch B=128 across 8 NeuronCores (16 items/core,
processed as 8 pairs with free-dim-512 matmuls for all shared-weight FCs).
All matmul operands bf16 (fp32 PSUM accumulate). Softmax is transpose-free:
the intra attention matrix is symmetric and the cross attention is computed
in both orientations by direct matmuls; attention-weight matmuls consume the
raw exp tiles and the reciprocal softmax denominators are applied at PSUM
drain time via a ones-outer-product broadcast. Sequence masks fold into the
exp scale column and the attended operand rows; pooling runs on the PE with
the mask column as lhsT. Per-pair work is emitted as a 6-stage software
pipeline so every cross-engine latency is covered by other pairs' matmuls.
"""
import sys
import numpy as np

for _p in ("/opt/trn_rl_repo",):
    if _p not in sys.path:
        sys.path.append(_p)

import ml_dtypes
import concourse.bass as bass
import concourse.bacc as bacc
import concourse.tile as tile
from concourse import mybir
from concourse.bass_utils import run_bass_kernel_spmd
from concourse.masks import make_identity

F32 = mybir.dt.float32
BF16 = mybir.dt.bfloat16
FP8 = mybir.dt.float8e4
I32 = mybir.dt.int32
AF = mybir.ActivationFunctionType
ALU = mybir.AluOpType
AX = mybir.AxisListType
BF_NP = ml_dtypes.bfloat16

L, EMB, PROJ, ATT, CLS = 256, 300, 200, 200, 3
B, NCORES = 128, 8
NIT = B // NCORES            # items per core
NPAIR = NIT // 2
VOCAB = 50000

D_SL = [(0, 128), (128, 256), (256, 300)]          # EMB k-tiles
A_SL = [(0, 128), (128, 200)]                      # ATT/PROJ tiles
WC_K = [(0, 128), (128, 200), (200, 328), (328, 400)]
V_CH = [(0, 128), (128, 256), (256, 384), (384, 400)]  # P transpose chunks

_CACHED_NC = None


def _build_nc():
    nc = bacc.Bacc("TRN2", target_bir_lowering=False, debug=False)

    dram = {}
    def din(name, shape, dt):
        dram[name] = nc.dram_tensor(name, shape, dt, kind="ExternalInput")
        return dram[name]

    din("idx1", [128, 2 * NIT], I32)
    din("idx2", [128, 2 * NIT], I32)
    din("xi1", [NIT, L], I32)
    din("xi2", [NIT, L], I32)
    din("emb", [VOCAB, EMB], BF16)
    din("wi3a0", [128, 2, 128], FP8)
    din("wi3a1", [128, 2, 128], FP8)
    din("wi_t", [44, ATT], BF16)
    din("wp3a0", [128, 2, 128], FP8)
    din("wp3a1", [128, 2, 128], FP8)
    din("wp_ta", [44, PROJ], BF16)
    din("wp3b0", [128, 2, 128], FP8)
    din("wp3b1", [128, 2, 128], FP8)
    din("wp_tb", [44, PROJ], BF16)
    din("wa", [PROJ, ATT], BF16)
    din("wc3_0", [128, 2, 2 * PROJ], FP8)
    din("wc3_1", [128, 2, 2 * PROJ], FP8)
    din("wg", [4 * PROJ, CLS], BF16)
    din("bi", [ATT, 1], F32)
    din("bp", [PROJ, 1], F32)
    din("bp16", [PROJ, 1], F32)
    din("ba_row", [1, ATT], BF16)
    din("bc_row", [1, 2 * PROJ], BF16)
    din("bg_row", [1, CLS], BF16)
    din("dmask", [L, L], BF16)
    din("bdist", [128, 1], F32)
    out_d = nc.dram_tensor("out", [CLS, NIT], F32, kind="ExternalOutput")

    with tile.TileContext(nc) as tc:
        _emit(nc, tc, dram, out_d)
    nc.compile()
    return nc


def _emit(nc, tc, dram, out_d):
    from contextlib import ExitStack
    ctx = ExitStack()
    with ctx:
        C = ctx.enter_context(tc.tile_pool(name="consts", bufs=1))
        PS = ctx.enter_context(tc.tile_pool(name="ps", bufs=7, space="PSUM"))
        W = ctx.enter_context(tc.tile_pool(name="work", bufs=3))

        def ps_tile(shape, dt=F32):
            return PS.tile(shape, dt, tag="ps", name="ps")

        def wtile(tag, shape=(128, 512), dt=BF16, bufs=3):
            return W.tile(list(shape), dt, tag=tag, name=tag, bufs=bufs)

        # ---------------- small input DMAs first ----------------
        idx_sb = {}
        for s, name in ((0, "idx1"), (1, "idx2")):
            t = C.tile([128, 2 * NIT], I32, tag=name, name=name)
            nc.sync.dma_start(out=t[:], in_=dram[name].ap())
            idx_sb[s] = t
        xi_sb = {}
        for s, name in ((0, "xi1"), (1, "xi2")):
            t = C.tile([NIT, L], I32, tag=name, name=name)
            nc.sync.dma_start(out=t[:], in_=dram[name].ap())
            xi_sb[s] = t

        dmask_d = dram["dmask"].ap()
        dmask_sb = []
        for (t0, t1) in [(0, 128), (128, 256)]:
            t = C.tile([128, 256], BF16, tag=f"dmask_{t0}", name=f"dmask_{t0}")
            nc.sync.dma_start(out=t[:t1 - t0, :], in_=dmask_d[t0:t1, :])
            dmask_sb.append(t)
        bdist = C.tile([128, 1], F32, tag="bdist", name="bdist")
        nc.sync.dma_start(out=bdist[:], in_=dram["bdist"].ap())

        # ---------------- constants ----------------
        ident_f = C.tile([128, 128], F32)
        make_identity(nc, ident_f[:])
        ident = C.tile([128, 128], BF16)
        nc.vector.tensor_copy(ident[:], ident_f[:])
        ones_bf = C.tile([1, 512], BF16)
        nc.vector.memset(ones_bf[:], 1.0)
        iota_i = C.tile([NIT, L], I32)
        nc.gpsimd.iota(iota_i[:], pattern=[[1, L]], base=0, channel_multiplier=0)
        iota16 = C.tile([NIT, L], F32)
        nc.vector.tensor_copy(iota16[:], iota_i[:])

        # ---------------- embedding gathers + squares (gpsimd) ----------------
        e_n = {}
        invsq = C.tile([128, 8 * NPAIR], F32)
        inv = C.tile([128, 8 * NPAIR], F32)

        def gcol(p, s, ti, h):
            return 8 * p + 4 * s + 2 * ti + h

        for p in range(NPAIR):
            for s in range(2):
                for ti in range(2):
                    t = C.tile([128, 600], BF16, tag=f"e_{s}_{ti}_{p}",
                               name=f"e_{s}_{ti}_{p}")
                    e_n[(s, ti, p)] = t
                    for h in range(2):
                        g = 2 * (2 * p + h) + ti
                        nc.gpsimd.indirect_dma_start(
                            out=t[:, h * 300:(h + 1) * 300], out_offset=None,
                            in_=dram["emb"].ap(),
                            in_offset=bass.IndirectOffsetOnAxis(
                                ap=idx_sb[s][:, g:g + 1], axis=0))

        # ---------------- masks ----------------
        m_bf = {}
        mcol_f = {}
        mcol_b = {}
        for s in range(2):
            xf = W.tile([NIT, L], F32, tag="xf", name="xf", bufs=1)
            nc.vector.tensor_copy(xf[:], xi_sb[s][:])
            nz = W.tile([NIT, L], F32, tag="nz", name="nz", bufs=1)
            nc.vector.tensor_scalar(nz[:], xf[:], 0.0, None, op0=ALU.not_equal)
            sizes = C.tile([NIT, 1], F32, tag=f"sizes{s}", name=f"sizes{s}")
            nc.vector.tensor_reduce(sizes[:], nz[:], axis=AX.X, op=ALU.add)
            mb = C.tile([NIT, L], BF16, tag=f"mbf{s}", name=f"mbf{s}")
            nc.vector.tensor_scalar(mb[:], iota16[:], sizes[:, :1], None,
                                    op0=ALU.is_lt)
            m_bf[s] = mb
            cf, cb = [], []
            for ti in range(2):
                pt = ps_tile([128, NIT], BF16)
                nc.tensor.transpose(pt[:, :NIT], mb[:NIT, ti * 128:(ti + 1) * 128],
                                    ident[:NIT, :NIT])
                f = C.tile([128, NIT], F32, tag=f"mcf{s}_{ti}", name=f"mcf{s}_{ti}")
                nc.vector.tensor_copy(f[:], pt[:, :NIT])
                bt = C.tile([128, NIT], BF16, tag=f"mcb{s}_{ti}", name=f"mcb{s}_{ti}")
                nc.scalar.copy(bt[:], pt[:, :NIT])
                cf.append(f)
                cb.append(bt)
            mcol_f[s] = cf
            mcol_b[s] = cb

        # ---------------- weights ----------------
        def load(name, r0, r1, dt=BF16):
            src = dram[name].ap()
            w = src.shape[1]
            t = C.tile([128, w], dt, tag=f"{name}_{r0}", name=f"{name}_{r0}")
            nc.sync.dma_start(out=t[:r1 - r0, :], in_=src[r0:r1, :])
            return t

        def load3(name, shape, dt=FP8):
            t = C.tile(list(shape), dt, tag=name, name=name)
            nc.sync.dma_start(out=t[:], in_=dram[name].ap())
            return t

        wi3 = [load3("wi3a0", (128, 2, 128)), load3("wi3a1", (128, 2, 128))]
        wi_t = load3("wi_t", (44, ATT), BF16)
        wp3a = [load3("wp3a0", (128, 2, 128)), load3("wp3a1", (128, 2, 128))]
        wp_ta = load3("wp_ta", (44, PROJ), BF16)
        wp3b = [load3("wp3b0", (128, 2, 128)), load3("wp3b1", (128, 2, 128))]
        wp_tb = load3("wp_tb", (44, PROJ), BF16)
        wc3 = [load3("wc3_0", (128, 2, 400)), load3("wc3_1", (128, 2, 400))]
        wa_k = [load("wa", a0, a1) for (a0, a1) in A_SL]
        wg_k = [load("wg", s * 400 + v0, s * 400 + v1)
                for s in range(2) for (v0, v1) in V_CH]
        bi_sl = [load("bi", a0, a1, F32) for (a0, a1) in A_SL]
        bp_sl = [load("bp", p0, p1, F32) for (p0, p1) in A_SL]
        bp16_sl = [load("bp16", p0, p1, F32) for (p0, p1) in A_SL]
        ba_row = load("ba_row", 0, 1)
        bc_row = load("bc_row", 0, 1)
        bg_row = load("bg_row", 0, 1)

        # bias2d pair tiles [x-blk, 512] bf16 = bdist * (dist>=10), duplicated
        bias2d = []
        for xb in range(2):
            b2 = C.tile([128, 512], BF16, tag=f"bias2d_{xb}", name=f"bias2d_{xb}")
            for h in range(2):
                nc.vector.tensor_scalar_mul(
                    b2[:, h * 256:(h + 1) * 256], dmask_sb[xb][:], bdist[:, :1])
            bias2d.append(b2)

        P_f = [C.tile([NIT, 400], F32, tag=f"P{s}", name=f"P{s}") for s in range(2)]

        # ---------------- pipeline stages ----------------
        state = {}

        def rden_pre(denst, prefix):
            """denst [128,4] f32 cols (h,blk) -> rrow [1,512] bf16 of
            reciprocal denominators (via PE column->row transposes)."""
            rden = wtile(f"{prefix}_rd", (128, 4), F32, bufs=4)
            nc.vector.reciprocal(rden[:], denst[:])
            rdbf = wtile(f"{prefix}_rdb", (128, 4), BF16, bufs=4)
            nc.vector.tensor_copy(rdbf[:], rden[:])
            rowps = ps_tile([1, 512], BF16)
            for h in range(2):
                for blk in range(2):
                    c = 2 * h + blk
                    nc.tensor.transpose(
                        rowps[:1, h * 256 + blk * 128: h * 256 + (blk + 1) * 128],
                        rdbf[:, c:c + 1], ident[:])
            rrow = wtile(f"{prefix}_rr", (1, 512), BF16, bufs=4)
            nc.scalar.copy(rrow[:], rowps[:1, :])
            return rrow

        def rden_bcast(rrow, prefix):
            bps = ps_tile([128, 512])
            nc.tensor.matmul(bps[:, :], lhsT=ones_bf[:1, :128], rhs=rrow[:1, :],
                             start=True, stop=True)
            rb = wtile(f"{prefix}_rb", (128, 512), BF16, bufs=2)
            nc.vector.tensor_copy(rb[:], bps[:, :])
            return rb

        def stage0(p):
            """norms (squares + Newton rsqrt + scale), eT, fT for both sides."""
            st = state.setdefault(p, {})
            for s in range(2):
                for ti in range(2):
                    for h in range(2):
                        g = gcol(p, s, ti, h)
                        scr = W.tile([128, 300], BF16, tag="sqscr", name="sqscr",
                                     bufs=4)
                        src_ap = e_n[(s, ti, p)][:, h * 300:(h + 1) * 300]
                        if (ti + h) % 2 == 0:
                            nc.scalar.activation(scr[:], src_ap, AF.Square,
                                                 accum_out=invsq[:, g:g + 1])
                        else:
                            nc.vector.scalar_tensor_tensor(
                                scr[:], src_ap, 1.0, src_ap,
                                op0=ALU.mult, op1=ALU.mult,
                                accum_out=invsq[:, g:g + 1])
            # Newton rsqrt on [128, 8]: magic seed + 2 iterations
            c0, c1 = 8 * p, 8 * p + 8
            x = invsq[:, c0:c1]
            it_ = wtile("nwt_i", (128, 8), I32, bufs=2)
            nc.vector.tensor_scalar(it_[:], x.bitcast(I32), 1, None,
                                    op0=ALU.arith_shift_right)
            nc.vector.tensor_scalar(it_[:], it_[:], -1, 0x5F3759DF,
                                    op0=ALU.mult, op1=ALU.add)
            y = it_[:].bitcast(F32)
            t1 = wtile("nwt_t", (128, 8), F32, bufs=2)
            for _ in range(2):
                nc.vector.tensor_mul(t1[:], y, y)
                nc.vector.tensor_mul(t1[:], t1[:], x)
                nc.vector.tensor_scalar(t1[:], t1[:], -0.5, 1.5,
                                        op0=ALU.mult, op1=ALU.add)
                nc.vector.tensor_mul(y, y, t1[:])
            nc.vector.tensor_copy(inv[:, c0:c1], y)
            for s in range(2):
                for ti in range(2):
                    for h in range(2):
                        g = gcol(p, s, ti, h)
                        t = e_n[(s, ti, p)]
                        nc.vector.tensor_scalar_mul(
                            t[:, h * 300:(h + 1) * 300],
                            t[:, h * 300:(h + 1) * 300], inv[:, g:g + 1])
            for s in range(2):
                e3 = wtile(f"eT3{s}", (128, 2, 512), FP8, bufs=3)
                et_t = wtile(f"eTt{s}", (128, 512), BF16, bufs=3)
                for di, (d0, d1) in enumerate(D_SL):
                    dsz = d1 - d0
                    tps = ps_tile([128, 512], BF16)
                    for h in range(2):
                        for ti in range(2):
                            nc.tensor.transpose(
                                tps[:dsz, h * 256 + ti * 128:
                                    h * 256 + (ti + 1) * 128],
                                e_n[(s, ti, p)][:, h * 300 + d0: h * 300 + d1],
                                ident[:])
                    if di < 2:
                        nc.vector.tensor_scalar(
                            e3[:, di, :], tps[:, :], 16.0, None, op0=ALU.mult)
                    else:
                        nc.vector.tensor_copy(et_t[:dsz, :], tps[:dsz, :])
                st[f"eT3{s}"] = e3
                st[f"eTt{s}"] = et_t
            for s in range(2):
                fT = []
                for ai, (a0, a1) in enumerate(A_SL):
                    asz = a1 - a0
                    ps = ps_tile([128, 512])
                    nc.tensor.matmul(ps[:, :],
                                     lhsT=wi3[ai][:, :, :],
                                     rhs=st[f"eT3{s}"][:, :, :],
                                     start=True, stop=False,
                                     perf_mode=mybir.MatmulPerfMode.DoubleRow)
                    nc.tensor.matmul(ps[:asz, :],
                                     lhsT=wi_t[:44, a0:a1],
                                     rhs=st[f"eTt{s}"][:44, :],
                                     start=False, stop=True)
                    t = wtile(f"fT{s}{ai}", bufs=2)
                    nc.scalar.activation(t[:asz, :], ps[:asz, :], AF.Relu,
                                         bias=bi_sl[ai][:asz, :1],
                                         scale=1.0 / 256.0)
                    fT.append(t)
                st[f"fT{s}"] = fT

        def stage1a(p):
            """att (+bias via identity matmul) and exps for both sides."""
            st = state[p]
            for s in range(2):
                fT = st[f"fT{s}"]
                denst = wtile(f"iden{s}", (128, 4), F32, bufs=3)
                E = []
                att_ps = []
                for xb in range(2):
                    ps = ps_tile([128, 512])
                    for h in range(2):
                        for ai, (a0, a1) in enumerate(A_SL):
                            asz = a1 - a0
                            nc.tensor.matmul(
                                ps[:, h * 256:(h + 1) * 256],
                                lhsT=fT[ai][:asz, h * 256 + xb * 128:
                                            h * 256 + (xb + 1) * 128],
                                rhs=fT[ai][:asz, h * 256:(h + 1) * 256],
                                start=(ai == 0), stop=False)
                        nc.tensor.matmul(
                            ps[:, h * 256:(h + 1) * 256],
                            lhsT=ident[:, :], rhs=bias2d[xb][:, h * 256:(h + 1) * 256],
                            start=False, stop=True)
                    att_ps.append(ps)
                for xb in range(2):
                    et = wtile(f"E{s}{xb}", bufs=2)
                    for h in range(2):
                        nc.scalar.activation(
                            et[:, h * 256:(h + 1) * 256],
                            att_ps[xb][:, h * 256:(h + 1) * 256], AF.Exp,
                            accum_out=denst[:, 2 * h + xb: 2 * h + xb + 1])
                    E.append(et)
                st[f"E{s}"] = E
                st[f"denI{s}"] = denst

        def stage1b(p):
            """rden prep, then per side: xp matmuls, broadcast, drains."""
            st = state[p]
            rrows = [rden_pre(st[f"denI{s}"], f"i{s}") for s in range(2)]
            for s in range(2):
                E = st[f"E{s}"]
                xp_ps = []
                for di, (d0, d1) in enumerate(D_SL):
                    dsz = d1 - d0
                    ps = ps_tile([128, 512])
                    for h in range(2):
                        for ti in range(2):
                            nc.tensor.matmul(
                                ps[:dsz, h * 256:(h + 1) * 256],
                                lhsT=e_n[(s, ti, p)][:, h * 300 + d0: h * 300 + d1],
                                rhs=E[ti][:, h * 256:(h + 1) * 256],
                                start=(ti == 0), stop=(ti == 1))
                    xp_ps.append(ps)
                rb = rden_bcast(rrows[s], f"i{s}")
                x3 = wtile(f"xp3{s}", (128, 2, 512), FP8, bufs=2)
                xt = wtile(f"xpt{s}", (128, 512), BF16, bufs=2)
                for di, (d0, d1) in enumerate(D_SL):
                    dsz = d1 - d0
                    if di < 2:
                        nc.vector.scalar_tensor_tensor(
                            x3[:, di, :], xp_ps[di][:, :], 16.0, rb[:, :],
                            op0=ALU.mult, op1=ALU.mult)
                    else:
                        nc.vector.tensor_mul(xt[:dsz, :], xp_ps[di][:dsz, :],
                                             rb[:dsz, :])
                st[f"xp3{s}"] = x3
                st[f"xpt{s}"] = xt

        def stage2(p):
            """pT, pRow, aT for both sides."""
            st = state[p]
            for s in range(2):
                pT = []
                kt3 = [wtile(f"kt3{s}{pi}", (128, 4, 2, 128), FP8, bufs=4)
                       for pi in range(2)]
                nc.gpsimd.memset(kt3[1][64:128, :, :, :], 0.0)
                for pi, (p0, p1) in enumerate(A_SL):
                    psz = p1 - p0
                    ps = ps_tile([128, 512])
                    nc.tensor.matmul(ps[:, :], lhsT=wp3a[pi][:, :, :],
                                     rhs=st[f"eT3{s}"][:, :, :],
                                     start=True, stop=False,
                                     perf_mode=mybir.MatmulPerfMode.DoubleRow)
                    nc.tensor.matmul(ps[:psz, :], lhsT=wp_ta[:44, p0:p1],
                                     rhs=st[f"eTt{s}"][:44, :],
                                     start=False, stop=False)
                    nc.tensor.matmul(ps[:, :], lhsT=wp3b[pi][:, :, :],
                                     rhs=st[f"xp3{s}"][:, :, :],
                                     start=False, stop=False,
                                     perf_mode=mybir.MatmulPerfMode.DoubleRow)
                    nc.tensor.matmul(ps[:psz, :], lhsT=wp_tb[:44, p0:p1],
                                     rhs=st[f"xpt{s}"][:44, :],
                                     start=False, stop=True)
                    t = wtile(f"pT{s}{pi}", bufs=2)
                    nc.scalar.activation(t[:psz, :], ps[:psz, :], AF.Identity,
                                         bias=bp_sl[pi][:psz, :1],
                                         scale=1.0 / 256.0)
                    nc.scalar.activation(kt3[pi][:psz, :, 0, :], ps[:psz, :],
                                         AF.Identity,
                                         bias=bp16_sl[pi][:psz, :1],
                                         scale=16.0 / 256.0)
                    pT.append(t)
                st[f"pT{s}"] = pT
                st[f"kt3{s}"] = kt3
            for s in range(2):
                pT = st[f"pT{s}"]
                pRow = []
                for ti in range(2):
                    tps = ps_tile([128, 400], BF16)
                    for h in range(2):
                        for pi, (p0, p1) in enumerate(A_SL):
                            psz = p1 - p0
                            nc.tensor.transpose(
                                tps[:, h * 200 + p0: h * 200 + p1],
                                pT[pi][:psz, h * 256 + ti * 128:
                                       h * 256 + (ti + 1) * 128],
                                ident[:psz, :psz])
                    t = wtile(f"pR{s}{ti}", (128, 400), bufs=3)
                    nc.scalar.copy(t[:], tps[:, :])
                    pRow.append(t)
                st[f"pR{s}"] = pRow
            maskrow = {}
            for s in range(2):
                rps = ps_tile([1, 512], BF16)
                for h in range(2):
                    it = 2 * p + h
                    for ti in range(2):
                        nc.tensor.transpose(
                            rps[:1, h * 256 + ti * 128: h * 256 + (ti + 1) * 128],
                            mcol_b[s][ti][:, it:it + 1], ident[:])
                mrow = wtile(f"mrow{s}", (1, 512), BF16, bufs=2)
                nc.vector.tensor_copy(mrow[:], rps[:1, :])
                bps = ps_tile([128, 512])
                nc.tensor.matmul(bps[:, :], lhsT=ones_bf[:1, :128],
                                 rhs=mrow[:1, :], start=True, stop=True)
                mr = wtile(f"mrowB{s}", (128, 512), BF16, bufs=2)
                nc.vector.tensor_copy(mr[:], bps[:, :])
                maskrow[s] = mr
            for s in range(2):
                pT = st[f"pT{s}"]
                aT = []
                for ai, (a0, a1) in enumerate(A_SL):
                    asz = a1 - a0
                    ps = ps_tile([128, 512])
                    for ki, (k0, k1) in enumerate(A_SL):
                        ksz = k1 - k0
                        nc.tensor.matmul(ps[:asz, :], lhsT=wa_k[ki][:ksz, a0:a1],
                                         rhs=pT[ki][:ksz, :], start=(ki == 0),
                                         stop=False)
                    nc.tensor.matmul(ps[:asz, :], lhsT=ba_row[:1, a0:a1],
                                     rhs=ones_bf[:1, :], start=False, stop=True)
                    t = wtile(f"aT{s}{ai}", bufs=2)
                    nc.vector.scalar_tensor_tensor(
                        t[:asz, :], ps[:asz, :], 0.0, maskrow[s][:asz, :],
                        op0=ALU.max, op1=ALU.mult)
                    aT.append(t)
                st[f"aT{s}"] = aT

        def stage3a(p):
            """sim & simT matmuls and exps with mask scale."""
            st = state[p]
            a1T, a2T = st["aT0"], st["aT1"]
            den2 = wtile("den2", (128, 4), F32, bufs=3)
            den1 = wtile("den1", (128, 4), F32, bufs=3)
            E2, E1 = [], []
            sim_ps, simT_ps = [], []
            for xb in range(2):
                ps = ps_tile([128, 512])
                for h in range(2):
                    for ai, (a0, a1) in enumerate(A_SL):
                        asz = a1 - a0
                        nc.tensor.matmul(
                            ps[:, h * 256:(h + 1) * 256],
                            lhsT=a1T[ai][:asz, h * 256 + xb * 128:
                                         h * 256 + (xb + 1) * 128],
                            rhs=a2T[ai][:asz, h * 256:(h + 1) * 256],
                            start=(ai == 0), stop=(ai == 1))
                sim_ps.append(ps)
            for xb in range(2):
                et = wtile(f"E2_{xb}", bufs=2)
                for h in range(2):
                    it = 2 * p + h
                    nc.scalar.activation(
                        et[:, h * 256:(h + 1) * 256],
                        sim_ps[xb][:, h * 256:(h + 1) * 256], AF.Exp,
                        scale=mcol_f[0][xb][:, it:it + 1],
                        accum_out=den2[:, 2 * h + xb: 2 * h + xb + 1])
                E2.append(et)
            for yb in range(2):
                ps = ps_tile([128, 512])
                for h in range(2):
                    for ai, (a0, a1) in enumerate(A_SL):
                        asz = a1 - a0
                        nc.tensor.matmul(
                            ps[:, h * 256:(h + 1) * 256],
                            lhsT=a2T[ai][:asz, h * 256 + yb * 128:
                                         h * 256 + (yb + 1) * 128],
                            rhs=a1T[ai][:asz, h * 256:(h + 1) * 256],
                            start=(ai == 0), stop=(ai == 1))
                simT_ps.append(ps)
            for yb in range(2):
                et = wtile(f"E1_{yb}", bufs=2)
                for h in range(2):
                    it = 2 * p + h
                    nc.scalar.activation(
                        et[:, h * 256:(h + 1) * 256],
                        simT_ps[yb][:, h * 256:(h + 1) * 256], AF.Exp,
                        scale=mcol_f[1][yb][:, it:it + 1],
                        accum_out=den1[:, 2 * h + yb: 2 * h + yb + 1])
                E1.append(et)
            st["E2"], st["E1"] = E2, E1
            st["den2"], st["den1"] = den2, den1

        def stage3b(p):
            """betaT / alphaT matmuls with drain-time normalization."""
            st = state[p]
            rr2 = rden_pre(st["den2"], "x2")
            beta_ps = []
            for pi, (p0, p1) in enumerate(A_SL):
                psz = p1 - p0
                ps = ps_tile([128, 512])
                for h in range(2):
                    for ti in range(2):
                        nc.tensor.matmul(
                            ps[:psz, h * 256:(h + 1) * 256],
                            lhsT=st["pR1"][ti][:, h * 200 + p0: h * 200 + p1],
                            rhs=st["E1"][ti][:, h * 256:(h + 1) * 256],
                            start=(ti == 0), stop=(ti == 1))
                beta_ps.append(ps)
            b2 = rden_bcast(rr2, "x2")
            for pi, (p0, p1) in enumerate(A_SL):
                psz = p1 - p0
                nc.vector.scalar_tensor_tensor(
                    st["kt30"][pi][:psz, :, 1, :], beta_ps[pi][:psz, :], 16.0,
                    b2[:psz, :], op0=ALU.mult, op1=ALU.mult)
            rr1 = rden_pre(st["den1"], "x1")
            alpha_ps = []
            for pi, (p0, p1) in enumerate(A_SL):
                psz = p1 - p0
                ps = ps_tile([128, 512])
                for h in range(2):
                    for xb in range(2):
                        nc.tensor.matmul(
                            ps[:psz, h * 256:(h + 1) * 256],
                            lhsT=st["pR0"][xb][:, h * 200 + p0: h * 200 + p1],
                            rhs=st["E2"][xb][:, h * 256:(h + 1) * 256],
                            start=(xb == 0), stop=(xb == 1))
                alpha_ps.append(ps)
            b1 = rden_bcast(rr1, "x1")
            for pi, (p0, p1) in enumerate(A_SL):
                psz = p1 - p0
                nc.vector.scalar_tensor_tensor(
                    st["kt31"][pi][:psz, :, 1, :], alpha_ps[pi][:psz, :], 16.0,
                    b1[:psz, :], op0=ALU.mult, op1=ALU.mult)

        def stage5(p):
            """compare + bias + relu + PE pooling + row stash."""
            st = state[p]
            for s in range(2):
                kt3 = st[f"kt3{s}"]
                for h in range(2):
                    it = 2 * p + h
                    vrs = []
                    for ti in range(2):
                        cps = ps_tile([128, 400])
                        for k in range(2):
                            nc.tensor.matmul(
                                cps[:, :],
                                lhsT=kt3[k][:, 2 * h + ti, :, :],
                                rhs=wc3[k][:, :, :],
                                start=(k == 0), stop=False,
                                perf_mode=mybir.MatmulPerfMode.DoubleRow)
                        nc.tensor.matmul(cps[:, :], lhsT=ones_bf[:1, :128],
                                         rhs=bc_row[:1, :400],
                                         start=False, stop=True)
                        vr = wtile("vr", (128, 400), BF16, bufs=3)
                        if ti == 0:
                            nc.vector.tensor_scalar(vr[:], cps[:, :],
                                                    1.0 / 256.0, 0.0,
                                                    op0=ALU.mult, op1=ALU.max)
                        else:
                            nc.scalar.activation(vr[:], cps[:, :], AF.Relu,
                                                 scale=1.0 / 256.0)
                        vrs.append(vr)
                    pps = ps_tile([1, 400])
                    for ti in range(2):
                        nc.tensor.matmul(pps[:1, :],
                                         lhsT=mcol_b[s][ti][:, it:it + 1],
                                         rhs=vrs[ti][:, :],
                                         start=(ti == 0), stop=(ti == 1))
                    prow = wtile("prow", (1, 400), F32, bufs=3)
                    if s == 0:
                        nc.scalar.copy(prow[:], pps[:1, :])
                    else:
                        nc.vector.tensor_copy(prow[:], pps[:1, :])
                    nc.sync.dma_start(out=P_f[s][it:it + 1, :], in_=prow[:1, :])
            del state[p]

        stages = [stage0, stage1a, stage1b, stage2, stage3a, stage3b, stage5]
        NST = len(stages)
        for t in range(NPAIR + NST - 1):
            for k in reversed(range(NST)):
                p = t - k
                if 0 <= p < NPAIR:
                    stages[k](p)

        # ---------------- aggregate ----------------
        PT_sb = []
        for s in range(2):
            pb = C.tile([NIT, 400], BF16, tag=f"Pb{s}", name=f"Pb{s}")
            nc.vector.tensor_copy(pb[:], P_f[s][:])
            for c, (c0, c1) in enumerate(V_CH):
                csz = c1 - c0
                tps = ps_tile([128, NIT], BF16)
                nc.tensor.transpose(tps[:csz, :NIT], pb[:NIT, c0:c1],
                                    ident[:NIT, :NIT])
                t = C.tile([128, NIT], BF16, tag=f"PT{s}_{c}", name=f"PT{s}_{c}")
                nc.scalar.copy(t[:csz, :], tps[:csz, :])
                PT_sb.append(t)
        aps = ps_tile([CLS, NIT])
        for k in range(8):
            ksz = V_CH[k % 4][1] - V_CH[k % 4][0]
            nc.tensor.matmul(aps[:, :], lhsT=wg_k[k][:ksz, :CLS],
                             rhs=PT_sb[k][:ksz, :], start=(k == 0), stop=False)
        nc.tensor.matmul(aps[:, :], lhsT=bg_row[:1, :CLS],
                         rhs=ones_bf[:1, :NIT], start=False, stop=True)
        out_sb = C.tile([CLS, NIT], F32)
        nc.scalar.copy(out_sb[:], aps[:, :])
        nc.sync.dma_start(out=out_d.ap(), in_=out_sb[:])


def _get_nc():
    global _CACHED_NC
    if _CACHED_NC is None:
        _CACHED_NC = _build_nc()
    return _CACHED_NC


def make_in_maps(inputs):
    x1 = np.asarray(inputs["x1"])
    x2 = np.asarray(inputs["x2"])
    f32 = lambda k: np.ascontiguousarray(np.asarray(inputs[k], dtype=np.float32))
    bf = lambda a: np.ascontiguousarray(np.asarray(a, dtype=np.float32)).astype(BF_NP)
    ii, jj = np.meshgrid(np.arange(L), np.arange(L), indexing="ij")
    dmask = (np.abs(ii - jj) >= 10).astype(np.float32)
    bdist = np.full((128, 1), np.asarray(inputs["b_dist"], np.float32).reshape(-1)[0],
                    np.float32)

    F8_NP = ml_dtypes.float8_e4m3fn
    q8 = lambda a: (np.ascontiguousarray(np.asarray(a, np.float32)) * 16.0).astype(F8_NP)
    Wi, Wp, Wa, Wc = (np.asarray(inputs[k], np.float32)
                      for k in ("Wi", "Wp", "Wa", "Wc"))

    def pack2(a, b, pad_n=None):
        # [K<=128, 2, N] fp8(x16), zero-padding ragged K and N
        K = 128
        N = pad_n or a.shape[1]
        out = np.zeros((K, 2, N), np.float32)
        out[:a.shape[0], 0, :a.shape[1]] = a
        out[:b.shape[0], 1, :b.shape[1]] = b
        return (out * 16.0).astype(F8_NP)

    shared = {
        "emb": bf(inputs["emb"]),
        "wi3a0": pack2(Wi[0:128, 0:128], Wi[128:256, 0:128]),
        "wi3a1": pack2(Wi[0:128, 128:200], Wi[128:256, 128:200], pad_n=128),
        "wi_t": bf(Wi[256:300] * 256.0),
        "wp3a0": pack2(Wp[0:128, 0:128], Wp[128:256, 0:128]),
        "wp3a1": pack2(Wp[0:128, 128:200], Wp[128:256, 128:200], pad_n=128),
        "wp_ta": bf(Wp[256:300] * 256.0),
        "wp3b0": pack2(Wp[300:428, 0:128], Wp[428:556, 0:128]),
        "wp3b1": pack2(Wp[300:428, 128:200], Wp[428:556, 128:200], pad_n=128),
        "wp_tb": bf(Wp[556:600] * 256.0),
        "wa": bf(Wa),
        "wc3_0": pack2(Wc[0:128], Wc[200:328]),
        "wc3_1": pack2(Wc[128:200], Wc[328:400]),
        "wg": bf(inputs["Wg"]),
        "bi": f32("bi").reshape(-1, 1), "bp": f32("bp").reshape(-1, 1),
        "bp16": f32("bp").reshape(-1, 1) * 16.0,
        "ba_row": bf(np.asarray(inputs["ba"]).reshape(1, -1)),
        "bc_row": bf(np.asarray(inputs["bc"]).reshape(1, -1) * 256.0),
        "bg_row": bf(np.asarray(inputs["bg"]).reshape(1, -1)),
        "dmask": dmask.astype(BF_NP), "bdist": bdist,
    }
    in_maps = []
    for c in range(NCORES):
        sl = slice(c * NIT, (c + 1) * NIT)
        x1s = np.ascontiguousarray(x1[sl]).astype(np.int32)
        x2s = np.ascontiguousarray(x2[sl]).astype(np.int32)
        m = dict(shared)
        m["idx1"] = np.ascontiguousarray(x1s.reshape(-1).reshape(2 * NIT, 128).T)
        m["idx2"] = np.ascontiguousarray(x2s.reshape(-1).reshape(2 * NIT, 128).T)
        m["xi1"] = x1s
        m["xi2"] = x2s
        in_maps.append(m)
    return in_maps


def kernel(**inputs):
    nc = _get_nc()
    in_maps = make_in_maps(inputs)
    res = run_bass_kernel_spmd(nc, in_maps, core_ids=list(range(NCORES)))
    out = np.concatenate([r["out"].T for r in res.results], axis=0)
    return np.ascontiguousarray(out, dtype=np.float32)


# revision 25
# speedup vs baseline: 1.0027x; 1.0027x over previous
"""Trainium2 Bass kernel for nn_Decomposeable (decomposable attention model).

Strategy: data-parallel over batch B=128 across 8 NeuronCores (16 items/core,
processed as 8 pairs with free-dim-512 matmuls for all shared-weight FCs).
All matmul operands bf16 (fp32 PSUM accumulate). Softmax is transpose-free:
the intra attention matrix is symmetric and the cross attention is computed
in both orientations by direct matmuls; attention-weight matmuls consume the
raw exp tiles and the reciprocal softmax denominators are applied at PSUM
drain time via a ones-outer-product broadcast. Sequence masks fold into the
exp scale column and the attended operand rows; pooling runs on the PE with
the mask column as lhsT. Per-pair work is emitted as a 6-stage software
pipeline so every cross-engine latency is covered by other pairs' matmuls.
"""
import sys
import numpy as np

for _p in ("/opt/trn_rl_repo",):
    if _p not in sys.path:
        sys.path.append(_p)

import ml_dtypes
import concourse.bass as bass
import concourse.bacc as bacc
import concourse.tile as tile
from concourse import mybir
from concourse.bass_utils import run_bass_kernel_spmd
from concourse.masks import make_identity

F32 = mybir.dt.float32
BF16 = mybir.dt.bfloat16
I32 = mybir.dt.int32
AF = mybir.ActivationFunctionType
ALU = mybir.AluOpType
AX = mybir.AxisListType
BF_NP = ml_dtypes.bfloat16

L, EMB, PROJ, ATT, CLS = 256, 300, 200, 200, 3
B, NCORES = 128, 8
NIT = B // NCORES            # items per core
NPAIR = NIT // 2
VOCAB = 50000

D_SL = [(0, 128), (128, 256), (256, 300)]          # EMB k-tiles
A_SL = [(0, 128), (128, 200)]                      # ATT/PROJ tiles
WC_K = [(0, 128), (128, 200), (200, 328), (328, 400)]
V_CH = [(0, 128), (128, 256), (256, 384), (384, 400)]  # P transpose chunks

_CACHED_NC = None


def _build_nc():
    nc = bacc.Bacc("TRN2", target_bir_lowering=False, debug=False)

    dram = {}
    def din(name, shape, dt):
        dram[name] = nc.dram_tensor(name, shape, dt, kind="ExternalInput")
        return dram[name]

    din("idx1", [128, 2 * NIT], I32)
    din("idx2", [128, 2 * NIT], I32)
    din("xi1", [NIT, L], I32)
    din("xi2", [NIT, L], I32)
    din("emb", [VOCAB, EMB], BF16)
    din("wi", [EMB, ATT], BF16)
    din("wp", [2 * EMB, PROJ], BF16)
    din("wa", [PROJ, ATT], BF16)
    din("wc", [2 * PROJ, 2 * PROJ], BF16)
    din("wg", [4 * PROJ, CLS], BF16)
    din("bi", [ATT, 1], F32)
    din("bp", [PROJ, 1], F32)
    din("ba_row", [1, ATT], BF16)
    din("bc_row", [1, 2 * PROJ], BF16)
    din("bg_row", [1, CLS], BF16)
    din("dmask", [L, L], BF16)
    din("bdist", [128, 1], F32)
    out_d = nc.dram_tensor("out", [CLS, NIT], F32, kind="ExternalOutput")

    with tile.TileContext(nc) as tc:
        _emit(nc, tc, dram, out_d)
    nc.compile()
    return nc


def _emit(nc, tc, dram, out_d):
    from contextlib import ExitStack
    ctx = ExitStack()
    with ctx:
        C = ctx.enter_context(tc.tile_pool(name="consts", bufs=1))
        PS = ctx.enter_context(tc.tile_pool(name="ps", bufs=7, space="PSUM"))
        W = ctx.enter_context(tc.tile_pool(name="work", bufs=3))

        def ps_tile(shape, dt=F32):
            return PS.tile(shape, dt, tag="ps", name="ps")

        def wtile(tag, shape=(128, 512), dt=BF16, bufs=3):
            return W.tile(list(shape), dt, tag=tag, name=tag, bufs=bufs)

        # ---------------- small input DMAs first ----------------
        idx_sb = {}
        for s, name in ((0, "idx1"), (1, "idx2")):
            t = C.tile([128, 2 * NIT], I32, tag=name, name=name)
            nc.sync.dma_start(out=t[:], in_=dram[name].ap())
            idx_sb[s] = t
        xi_sb = {}
        for s, name in ((0, "xi1"), (1, "xi2")):
            t = C.tile([NIT, L], I32, tag=name, name=name)
            nc.sync.dma_start(out=t[:], in_=dram[name].ap())
            xi_sb[s] = t

        dmask_d = dram["dmask"].ap()
        dmask_sb = []
        for (t0, t1) in [(0, 128), (128, 256)]:
            t = C.tile([128, 256], BF16, tag=f"dmask_{t0}", name=f"dmask_{t0}")
            nc.sync.dma_start(out=t[:t1 - t0, :], in_=dmask_d[t0:t1, :])
            dmask_sb.append(t)
        bdist = C.tile([128, 1], F32, tag="bdist", name="bdist")
        nc.sync.dma_start(out=bdist[:], in_=dram["bdist"].ap())

        # ---------------- constants ----------------
        ident_f = C.tile([128, 128], F32)
        make_identity(nc, ident_f[:])
        ident = C.tile([128, 128], BF16)
        nc.vector.tensor_copy(ident[:], ident_f[:])
        ones_bf = C.tile([1, 512], BF16)
        nc.vector.memset(ones_bf[:], 1.0)
        iota_i = C.tile([NIT, L], I32)
        nc.gpsimd.iota(iota_i[:], pattern=[[1, L]], base=0, channel_multiplier=0)
        iota16 = C.tile([NIT, L], F32)
        nc.vector.tensor_copy(iota16[:], iota_i[:])

        # ---------------- embedding gathers + squares (gpsimd) ----------------
        e_n = {}
        invsq = C.tile([128, 8 * NPAIR], F32)
        inv = C.tile([128, 8 * NPAIR], F32)

        def gcol(p, s, ti, h):
            return 8 * p + 4 * s + 2 * ti + h

        for p in range(NPAIR):
            for s in range(2):
                for ti in range(2):
                    t = C.tile([128, 600], BF16, tag=f"e_{s}_{ti}_{p}",
                               name=f"e_{s}_{ti}_{p}")
                    e_n[(s, ti, p)] = t
                    for h in range(2):
                        g = 2 * (2 * p + h) + ti
                        nc.gpsimd.indirect_dma_start(
                            out=t[:, h * 300:(h + 1) * 300], out_offset=None,
                            in_=dram["emb"].ap(),
                            in_offset=bass.IndirectOffsetOnAxis(
                                ap=idx_sb[s][:, g:g + 1], axis=0))

        # ---------------- masks ----------------
        m_bf = {}
        mcol_f = {}
        mcol_b = {}
        for s in range(2):
            xf = W.tile([NIT, L], F32, tag="xf", name="xf", bufs=1)
            nc.vector.tensor_copy(xf[:], xi_sb[s][:])
            nz = W.tile([NIT, L], F32, tag="nz", name="nz", bufs=1)
            nc.vector.tensor_scalar(nz[:], xf[:], 0.0, None, op0=ALU.not_equal)
            sizes = C.tile([NIT, 1], F32, tag=f"sizes{s}", name=f"sizes{s}")
            nc.vector.tensor_reduce(sizes[:], nz[:], axis=AX.X, op=ALU.add)
            mb = C.tile([NIT, L], BF16, tag=f"mbf{s}", name=f"mbf{s}")
            nc.vector.tensor_scalar(mb[:], iota16[:], sizes[:, :1], None,
                                    op0=ALU.is_lt)
            m_bf[s] = mb
            cf, cb = [], []
            for ti in range(2):
                pt = ps_tile([128, NIT], BF16)
                nc.tensor.transpose(pt[:, :NIT], mb[:NIT, ti * 128:(ti + 1) * 128],
                                    ident[:NIT, :NIT])
                f = C.tile([128, NIT], F32, tag=f"mcf{s}_{ti}", name=f"mcf{s}_{ti}")
                nc.vector.tensor_copy(f[:], pt[:, :NIT])
                bt = C.tile([128, NIT], BF16, tag=f"mcb{s}_{ti}", name=f"mcb{s}_{ti}")
                nc.scalar.copy(bt[:], pt[:, :NIT])
                cf.append(f)
                cb.append(bt)
            mcol_f[s] = cf
            mcol_b[s] = cb

        # ---------------- weights ----------------
        def load(name, r0, r1, dt=BF16):
            src = dram[name].ap()
            w = src.shape[1]
            t = C.tile([128, w], dt, tag=f"{name}_{r0}", name=f"{name}_{r0}")
            nc.sync.dma_start(out=t[:r1 - r0, :], in_=src[r0:r1, :])
            return t

        wi_k = [load("wi", d0, d1) for (d0, d1) in D_SL]
        wp_k = [load("wp", d0, d1) for (d0, d1) in D_SL] + \
               [load("wp", 300 + d0, 300 + d1) for (d0, d1) in D_SL]
        wa_k = [load("wa", a0, a1) for (a0, a1) in A_SL]
        wc_k = [load("wc", k0, k1) for (k0, k1) in WC_K]
        wg_k = [load("wg", s * 400 + v0, s * 400 + v1)
                for s in range(2) for (v0, v1) in V_CH]
        bi_sl = [load("bi", a0, a1, F32) for (a0, a1) in A_SL]
        bp_sl = [load("bp", p0, p1, F32) for (p0, p1) in A_SL]
        ba_row = load("ba_row", 0, 1)
        bc_row = load("bc_row", 0, 1)
        bg_row = load("bg_row", 0, 1)

        # bias2d pair tiles [x-blk, 512] bf16 = bdist * (dist>=10), duplicated
        bias2d = []
        for xb in range(2):
            b2 = C.tile([128, 512], BF16, tag=f"bias2d_{xb}", name=f"bias2d_{xb}")
            for h in range(2):
                nc.vector.tensor_scalar_mul(
                    b2[:, h * 256:(h + 1) * 256], dmask_sb[xb][:], bdist[:, :1])
            bias2d.append(b2)

        P_f = [C.tile([NIT, 400], F32, tag=f"P{s}", name=f"P{s}") for s in range(2)]

        # ---------------- pipeline stages ----------------
        state = {}

        def rden_pre(denst, prefix):
            """denst [128,4] f32 cols (h,blk) -> rrow [1,512] bf16 of
            reciprocal denominators (via PE column->row transposes)."""
            rden = wtile(f"{prefix}_rd", (128, 4), F32, bufs=4)
            nc.vector.reciprocal(rden[:], denst[:])
            rdbf = wtile(f"{prefix}_rdb", (128, 4), BF16, bufs=4)
            nc.vector.tensor_copy(rdbf[:], rden[:])
            rowps = ps_tile([1, 512], BF16)
            for h in range(2):
                for blk in range(2):
                    c = 2 * h + blk
                    nc.tensor.transpose(
                        rowps[:1, h * 256 + blk * 128: h * 256 + (blk + 1) * 128],
                        rdbf[:, c:c + 1], ident[:])
            rrow = wtile(f"{prefix}_rr", (1, 512), BF16, bufs=4)
            nc.scalar.copy(rrow[:], rowps[:1, :])
            return rrow

        def rden_bcast(rrow, prefix):
            bps = ps_tile([128, 512])
            nc.tensor.matmul(bps[:, :], lhsT=ones_bf[:1, :128], rhs=rrow[:1, :],
                             start=True, stop=True)
            rb = wtile(f"{prefix}_rb", (128, 512), BF16, bufs=2)
            nc.vector.tensor_copy(rb[:], bps[:, :])
            return rb

        def stage0(p):
            """norms (squares + Newton rsqrt + scale), eT, fT for both sides."""
            st = state.setdefault(p, {})
            for s in range(2):
                for ti in range(2):
                    for h in range(2):
                        g = gcol(p, s, ti, h)
                        scr = W.tile([128, 300], BF16, tag="sqscr", name="sqscr",
                                     bufs=4)
                        src_ap = e_n[(s, ti, p)][:, h * 300:(h + 1) * 300]
                        if (ti + h) % 2 == 0:
                            nc.scalar.activation(scr[:], src_ap, AF.Square,
                                                 accum_out=invsq[:, g:g + 1])
                        else:
                            nc.vector.scalar_tensor_tensor(
                                scr[:], src_ap, 1.0, src_ap,
                                op0=ALU.mult, op1=ALU.mult,
                                accum_out=invsq[:, g:g + 1])
            # Newton rsqrt on [128, 8]: magic seed + 2 iterations
            c0, c1 = 8 * p, 8 * p + 8
            x = invsq[:, c0:c1]
            it_ = wtile("nwt_i", (128, 8), I32, bufs=2)
            nc.vector.tensor_scalar(it_[:], x.bitcast(I32), 1, None,
                                    op0=ALU.arith_shift_right)
            nc.vector.tensor_scalar(it_[:], it_[:], -1, 0x5F3759DF,
                                    op0=ALU.mult, op1=ALU.add)
            y = it_[:].bitcast(F32)
            t1 = wtile("nwt_t", (128, 8), F32, bufs=2)
            for _ in range(2):
                nc.vector.tensor_mul(t1[:], y, y)
                nc.vector.tensor_mul(t1[:], t1[:], x)
                nc.vector.tensor_scalar(t1[:], t1[:], -0.5, 1.5,
                                        op0=ALU.mult, op1=ALU.add)
                nc.vector.tensor_mul(y, y, t1[:])
            nc.vector.tensor_copy(inv[:, c0:c1], y)
            for s in range(2):
                for ti in range(2):
                    for h in range(2):
                        g = gcol(p, s, ti, h)
                        t = e_n[(s, ti, p)]
                        nc.vector.tensor_scalar_mul(
                            t[:, h * 300:(h + 1) * 300],
                            t[:, h * 300:(h + 1) * 300], inv[:, g:g + 1])
            for s in range(2):
                eT = []
                for di, (d0, d1) in enumerate(D_SL):
                    dsz = d1 - d0
                    tps = ps_tile([128, 512], BF16)
                    for h in range(2):
                        for ti in range(2):
                            nc.tensor.transpose(
                                tps[:dsz, h * 256 + ti * 128:
                                    h * 256 + (ti + 1) * 128],
                                e_n[(s, ti, p)][:, h * 300 + d0: h * 300 + d1],
                                ident[:])
                    t = wtile(f"eT{s}{di}", bufs=3)
                    nc.vector.tensor_copy(t[:dsz, :], tps[:dsz, :])
                    eT.append(t)
                st[f"eT{s}"] = eT
            for s in range(2):
                fT = []
                for ai, (a0, a1) in enumerate(A_SL):
                    asz = a1 - a0
                    ps = ps_tile([128, 512])
                    for k in range(3):
                        ksz = D_SL[k][1] - D_SL[k][0]
                        nc.tensor.matmul(ps[:asz, :],
                                         lhsT=wi_k[k][:ksz, a0:a1],
                                         rhs=st[f"eT{s}"][k][:ksz, :],
                                         start=(k == 0), stop=(k == 2))
                    t = wtile(f"fT{s}{ai}", bufs=2)
                    nc.scalar.activation(t[:asz, :], ps[:asz, :], AF.Relu,
                                         bias=bi_sl[ai][:asz, :1])
                    fT.append(t)
                st[f"fT{s}"] = fT

        def stage1a(p):
            """att (+bias via identity matmul) and exps for both sides."""
            st = state[p]
            for s in range(2):
                fT = st[f"fT{s}"]
                denst = wtile(f"iden{s}", (128, 4), F32, bufs=3)
                E = []
                att_ps = []
                for xb in range(2):
                    ps = ps_tile([128, 512])
                    for h in range(2):
                        for ai, (a0, a1) in enumerate(A_SL):
                            asz = a1 - a0
                            nc.tensor.matmul(
                                ps[:, h * 256:(h + 1) * 256],
                                lhsT=fT[ai][:asz, h * 256 + xb * 128:
                                            h * 256 + (xb + 1) * 128],
                                rhs=fT[ai][:asz, h * 256:(h + 1) * 256],
                                start=(ai == 0), stop=False)
                        nc.tensor.matmul(
                            ps[:, h * 256:(h + 1) * 256],
                            lhsT=ident[:, :], rhs=bias2d[xb][:, h * 256:(h + 1) * 256],
                            start=False, stop=True)
                    att_ps.append(ps)
                for xb in range(2):
                    et = wtile(f"E{s}{xb}", bufs=2)
                    for h in range(2):
                        nc.scalar.activation(
                            et[:, h * 256:(h + 1) * 256],
                            att_ps[xb][:, h * 256:(h + 1) * 256], AF.Exp,
                            accum_out=denst[:, 2 * h + xb: 2 * h + xb + 1])
                    E.append(et)
                st[f"E{s}"] = E
                st[f"denI{s}"] = denst

        def stage1b(p):
            """rden prep, then per side: xp matmuls, broadcast, drains."""
            st = state[p]
            rrows = [rden_pre(st[f"denI{s}"], f"i{s}") for s in range(2)]
            for s in range(2):
                E = st[f"E{s}"]
                xp_ps = []
                for di, (d0, d1) in enumerate(D_SL):
                    dsz = d1 - d0
                    ps = ps_tile([128, 512])
                    for h in range(2):
                        for ti in range(2):
                            nc.tensor.matmul(
                                ps[:dsz, h * 256:(h + 1) * 256],
                                lhsT=e_n[(s, ti, p)][:, h * 300 + d0: h * 300 + d1],
                                rhs=E[ti][:, h * 256:(h + 1) * 256],
                                start=(ti == 0), stop=(ti == 1))
                    xp_ps.append(ps)
                rb = rden_bcast(rrows[s], f"i{s}")
                xpT = []
                for di, (d0, d1) in enumerate(D_SL):
                    dsz = d1 - d0
                    t = wtile(f"xp{s}{di}", bufs=2)
                    nc.vector.tensor_mul(t[:dsz, :], xp_ps[di][:dsz, :],
                                         rb[:dsz, :])
                    xpT.append(t)
                st[f"xp{s}"] = xpT

        def stage2(p):
            """pT, pRow, aT for both sides."""
            st = state[p]
            for s in range(2):
                hT = st[f"eT{s}"] + st[f"xp{s}"]
                pT = []
                for pi, (p0, p1) in enumerate(A_SL):
                    psz = p1 - p0
                    ps = ps_tile([128, 512])
                    for k in range(6):
                        ksz = D_SL[k % 3][1] - D_SL[k % 3][0]
                        nc.tensor.matmul(ps[:psz, :], lhsT=wp_k[k][:ksz, p0:p1],
                                         rhs=hT[k][:ksz, :], start=(k == 0),
                                         stop=(k == 5))
                    t = wtile(f"pT{s}{pi}", bufs=3)
                    nc.scalar.activation(t[:psz, :], ps[:psz, :], AF.Identity,
                                         bias=bp_sl[pi][:psz, :1])
                    pT.append(t)
                st[f"pT{s}"] = pT
            for s in range(2):
                pT = st[f"pT{s}"]
                pRow = []
                for ti in range(2):
                    tps = ps_tile([128, 400], BF16)
                    for h in range(2):
                        for pi, (p0, p1) in enumerate(A_SL):
                            psz = p1 - p0
                            nc.tensor.transpose(
                                tps[:, h * 200 + p0: h * 200 + p1],
                                pT[pi][:psz, h * 256 + ti * 128:
                                       h * 256 + (ti + 1) * 128],
                                ident[:psz, :psz])
                    t = wtile(f"pR{s}{ti}", (128, 400), bufs=3)
                    nc.scalar.copy(t[:], tps[:, :])
                    pRow.append(t)
                st[f"pR{s}"] = pRow
            maskrow = {}
            for s in range(2):
                rps = ps_tile([1, 512], BF16)
                for h in range(2):
                    it = 2 * p + h
                    for ti in range(2):
                        nc.tensor.transpose(
                            rps[:1, h * 256 + ti * 128: h * 256 + (ti + 1) * 128],
                            mcol_b[s][ti][:, it:it + 1], ident[:])
                mrow = wtile(f"mrow{s}", (1, 512), BF16, bufs=2)
                nc.vector.tensor_copy(mrow[:], rps[:1, :])
                bps = ps_tile([128, 512])
                nc.tensor.matmul(bps[:, :], lhsT=ones_bf[:1, :128],
                                 rhs=mrow[:1, :], start=True, stop=True)
                mr = wtile(f"mrowB{s}", (128, 512), BF16, bufs=2)
                nc.vector.tensor_copy(mr[:], bps[:, :])
                maskrow[s] = mr
            for s in range(2):
                pT = st[f"pT{s}"]
                aT = []
                for ai, (a0, a1) in enumerate(A_SL):
                    asz = a1 - a0
                    ps = ps_tile([128, 512])
                    for ki, (k0, k1) in enumerate(A_SL):
                        ksz = k1 - k0
                        nc.tensor.matmul(ps[:asz, :], lhsT=wa_k[ki][:ksz, a0:a1],
                                         rhs=pT[ki][:ksz, :], start=(ki == 0),
                                         stop=False)
                    nc.tensor.matmul(ps[:asz, :], lhsT=ba_row[:1, a0:a1],
                                     rhs=ones_bf[:1, :], start=False, stop=True)
                    t = wtile(f"aT{s}{ai}", bufs=2)
                    nc.vector.scalar_tensor_tensor(
                        t[:asz, :], ps[:asz, :], 0.0, maskrow[s][:asz, :],
                        op0=ALU.max, op1=ALU.mult)
                    aT.append(t)
                st[f"aT{s}"] = aT

        def stage3a(p):
            """sim & simT matmuls and exps with mask scale."""
            st = state[p]
            a1T, a2T = st["aT0"], st["aT1"]
            den2 = wtile("den2", (128, 4), F32, bufs=3)
            den1 = wtile("den1", (128, 4), F32, bufs=3)
            E2, E1 = [], []
            sim_ps, simT_ps = [], []
            for xb in range(2):
                ps = ps_tile([128, 512])
                for h in range(2):
                    for ai, (a0, a1) in enumerate(A_SL):
                        asz = a1 - a0
                        nc.tensor.matmul(
                            ps[:, h * 256:(h + 1) * 256],
                            lhsT=a1T[ai][:asz, h * 256 + xb * 128:
                                         h * 256 + (xb + 1) * 128],
                            rhs=a2T[ai][:asz, h * 256:(h + 1) * 256],
                            start=(ai == 0), stop=(ai == 1))
                sim_ps.append(ps)
            for xb in range(2):
                et = wtile(f"E2_{xb}", bufs=2)
                for h in range(2):
                    it = 2 * p + h
                    nc.scalar.activation(
                        et[:, h * 256:(h + 1) * 256],
                        sim_ps[xb][:, h * 256:(h + 1) * 256], AF.Exp,
                        scale=mcol_f[0][xb][:, it:it + 1],
                        accum_out=den2[:, 2 * h + xb: 2 * h + xb + 1])
                E2.append(et)
            for yb in range(2):
                ps = ps_tile([128, 512])
                for h in range(2):
                    for ai, (a0, a1) in enumerate(A_SL):
                        asz = a1 - a0
                        nc.tensor.matmul(
                            ps[:, h * 256:(h + 1) * 256],
                            lhsT=a2T[ai][:asz, h * 256 + yb * 128:
                                         h * 256 + (yb + 1) * 128],
                            rhs=a1T[ai][:asz, h * 256:(h + 1) * 256],
                            start=(ai == 0), stop=(ai == 1))
                simT_ps.append(ps)
            for yb in range(2):
                et = wtile(f"E1_{yb}", bufs=2)
                for h in range(2):
                    it = 2 * p + h
                    nc.scalar.activation(
                        et[:, h * 256:(h + 1) * 256],
                        simT_ps[yb][:, h * 256:(h + 1) * 256], AF.Exp,
                        scale=mcol_f[1][yb][:, it:it + 1],
                        accum_out=den1[:, 2 * h + yb: 2 * h + yb + 1])
                E1.append(et)
            st["E2"], st["E1"] = E2, E1
            st["den2"], st["den1"] = den2, den1

        def stage3b(p):
            """betaT / alphaT matmuls with drain-time normalization."""
            st = state[p]
            rr2 = rden_pre(st["den2"], "x2")
            betaT, alphaT = [], []
            beta_ps = []
            for pi, (p0, p1) in enumerate(A_SL):
                psz = p1 - p0
                ps = ps_tile([128, 512])
                for h in range(2):
                    for ti in range(2):
                        nc.tensor.matmul(
                            ps[:psz, h * 256:(h + 1) * 256],
                            lhsT=st["pR1"][ti][:, h * 200 + p0: h * 200 + p1],
                            rhs=st["E1"][ti][:, h * 256:(h + 1) * 256],
                            start=(ti == 0), stop=(ti == 1))
                beta_ps.append(ps)
            b2 = rden_bcast(rr2, "x2")
            for pi, (p0, p1) in enumerate(A_SL):
                psz = p1 - p0
                t = wtile(f"bT{pi}", bufs=2)
                nc.vector.tensor_mul(t[:psz, :], beta_ps[pi][:psz, :],
                                     b2[:psz, :])
                betaT.append(t)
            rr1 = rden_pre(st["den1"], "x1")
            alpha_ps = []
            for pi, (p0, p1) in enumerate(A_SL):
                psz = p1 - p0
                ps = ps_tile([128, 512])
                for h in range(2):
                    for xb in range(2):
                        nc.tensor.matmul(
                            ps[:psz, h * 256:(h + 1) * 256],
                            lhsT=st["pR0"][xb][:, h * 200 + p0: h * 200 + p1],
                            rhs=st["E2"][xb][:, h * 256:(h + 1) * 256],
                            start=(xb == 0), stop=(xb == 1))
                alpha_ps.append(ps)
            b1 = rden_bcast(rr1, "x1")
            for pi, (p0, p1) in enumerate(A_SL):
                psz = p1 - p0
                t = wtile(f"alT{pi}", bufs=2)
                nc.vector.tensor_mul(t[:psz, :], alpha_ps[pi][:psz, :],
                                     b1[:psz, :])
                alphaT.append(t)
            st["betaT"], st["alphaT"] = betaT, alphaT

        def stage5(p):
            """compare + bias + relu + PE pooling + row stash."""
            st = state[p]
            for s, pTt, oT in ((0, st["pT0"], st["betaT"]),
                               (1, st["pT1"], st["alphaT"])):
                kt = pTt + oT
                for h in range(2):
                    it = 2 * p + h
                    vrs = []
                    for ti in range(2):
                        cps = ps_tile([128, 400])
                        for k in range(4):
                            ksz = WC_K[k][1] - WC_K[k][0]
                            nc.tensor.matmul(
                                cps[:, :],
                                lhsT=kt[k][:ksz, h * 256 + ti * 128:
                                           h * 256 + (ti + 1) * 128],
                                rhs=wc_k[k][:ksz, :400],
                                start=(k == 0), stop=False)
                        nc.tensor.matmul(cps[:, :], lhsT=ones_bf[:1, :128],
                                         rhs=bc_row[:1, :400],
                                         start=False, stop=True)
                        vr = wtile("vr", (128, 400), BF16, bufs=3)
                        if ti == 0:
                            nc.vector.tensor_scalar(vr[:], cps[:, :], 0.0, None,
                                                    op0=ALU.max)
                        else:
                            nc.scalar.activation(vr[:], cps[:, :], AF.Relu)
                        vrs.append(vr)
                    pps = ps_tile([1, 400])
                    for ti in range(2):
                        nc.tensor.matmul(pps[:1, :],
                                         lhsT=mcol_b[s][ti][:, it:it + 1],
                                         rhs=vrs[ti][:, :],
                                         start=(ti == 0), stop=(ti == 1))
                    prow = wtile("prow", (1, 400), F32, bufs=3)
                    if s == 0:
                        nc.scalar.copy(prow[:], pps[:1, :])
                    else:
                        nc.vector.tensor_copy(prow[:], pps[:1, :])
                    nc.sync.dma_start(out=P_f[s][it:it + 1, :], in_=prow[:1, :])
            del state[p]

        stages = [stage0, stage1a, stage1b, stage2, stage3a, stage3b, stage5]
        NST = len(stages)
        for t in range(NPAIR + NST - 1):
            for k in reversed(range(NST)):
                p = t - k
                if 0 <= p < NPAIR:
                    stages[k](p)

        # ---------------- aggregate ----------------
        PT_sb = []
        for s in range(2):
            pb = C.tile([NIT, 400], BF16, tag=f"Pb{s}", name=f"Pb{s}")
            nc.vector.tensor_copy(pb[:], P_f[s][:])
            for c, (c0, c1) in enumerate(V_CH):
                csz = c1 - c0
                tps = ps_tile([128, NIT], BF16)
                nc.tensor.transpose(tps[:csz, :NIT], pb[:NIT, c0:c1],
                                    ident[:NIT, :NIT])
                t = C.tile([128, NIT], BF16, tag=f"PT{s}_{c}", name=f"PT{s}_{c}")
                nc.scalar.copy(t[:csz, :], tps[:csz, :])
                PT_sb.append(t)
        aps = ps_tile([CLS, NIT])
        for k in range(8):
            ksz = V_CH[k % 4][1] - V_CH[k % 4][0]
            nc.tensor.matmul(aps[:, :], lhsT=wg_k[k][:ksz, :CLS],
                             rhs=PT_sb[k][:ksz, :], start=(k == 0), stop=False)
        nc.tensor.matmul(aps[:, :], lhsT=bg_row[:1, :CLS],
                         rhs=ones_bf[:1, :NIT], start=False, stop=True)
        out_sb = C.tile([CLS, NIT], F32)
        nc.scalar.copy(out_sb[:], aps[:, :])
        nc.sync.dma_start(out=out_d.ap(), in_=out_sb[:])


def _get_nc():
    global _CACHED_NC
    if _CACHED_NC is None:
        _CACHED_NC = _build_nc()
    return _CACHED_NC


def make_in_maps(inputs):
    x1 = np.asarray(inputs["x1"])
    x2 = np.asarray(inputs["x2"])
    f32 = lambda k: np.ascontiguousarray(np.asarray(inputs[k], dtype=np.float32))
    bf = lambda a: np.ascontiguousarray(np.asarray(a, dtype=np.float32)).astype(BF_NP)
    ii, jj = np.meshgrid(np.arange(L), np.arange(L), indexing="ij")
    dmask = (np.abs(ii - jj) >= 10).astype(np.float32)
    bdist = np.full((128, 1), np.asarray(inputs["b_dist"], np.float32).reshape(-1)[0],
                    np.float32)

    shared = {
        "emb": bf(inputs["emb"]),
        "wi": bf(inputs["Wi"]), "wp": bf(inputs["Wp"]), "wa": bf(inputs["Wa"]),
        "wc": bf(inputs["Wc"]), "wg": bf(inputs["Wg"]),
        "bi": f32("bi").reshape(-1, 1), "bp": f32("bp").reshape(-1, 1),
        "ba_row": bf(np.asarray(inputs["ba"]).reshape(1, -1)),
        "bc_row": bf(np.asarray(inputs["bc"]).reshape(1, -1)),
        "bg_row": bf(np.asarray(inputs["bg"]).reshape(1, -1)),
        "dmask": dmask.astype(BF_NP), "bdist": bdist,
    }
    in_maps = []
    for c in range(NCORES):
        sl = slice(c * NIT, (c + 1) * NIT)
        x1s = np.ascontiguousarray(x1[sl]).astype(np.int32)
        x2s = np.ascontiguousarray(x2[sl]).astype(np.int32)
        m = dict(shared)
        m["idx1"] = np.ascontiguousarray(x1s.reshape(-1).reshape(2 * NIT, 128).T)
        m["idx2"] = np.ascontiguousarray(x2s.reshape(-1).reshape(2 * NIT, 128).T)
        m["xi1"] = x1s
        m["xi2"] = x2s
        in_maps.append(m)
    return in_maps


def kernel(**inputs):
    nc = _get_nc()
    in_maps = make_in_maps(inputs)
    res = run_bass_kernel_spmd(nc, in_maps, core_ids=list(range(NCORES)))
    out = np.concatenate([r["out"].T for r in res.results], axis=0)
    return np.ascontiguousarray(out, dtype=np.float32)


# revision 26
# speedup vs baseline: 1.0042x; 1.0015x over previous
"""Trainium2 Bass kernel for nn_Decomposeable (decomposable attention model).

Strategy: data-parallel over batch B=128 across 8 NeuronCores (16 items/core,
processed as 8 pairs with free-dim-512 matmuls for all shared-weight FCs).
All matmul operands bf16 (fp32 PSUM accumulate). Softmax is transpose-free:
the intra attention matrix is symmetric and the cross attention is computed
in both orientations by direct matmuls; attention-weight matmuls consume the
raw exp tiles and the reciprocal softmax denominators are applied at PSUM
drain time via a ones-outer-product broadcast. Sequence masks fold into the
exp scale column and the attended operand rows; pooling runs on the PE with
the mask column as lhsT. Per-pair work is emitted as a 6-stage software
pipeline so every cross-engine latency is covered by other pairs' matmuls.
"""
import sys
import numpy as np

for _p in ("/opt/trn_rl_repo",):
    if _p not in sys.path:
        sys.path.append(_p)

import ml_dtypes
import concourse.bass as bass
import concourse.bacc as bacc
import concourse.tile as tile
from concourse import mybir
from concourse.bass_utils import run_bass_kernel_spmd
from concourse.masks import make_identity

F32 = mybir.dt.float32
BF16 = mybir.dt.bfloat16
I32 = mybir.dt.int32
AF = mybir.ActivationFunctionType
ALU = mybir.AluOpType
AX = mybir.AxisListType
BF_NP = ml_dtypes.bfloat16

L, EMB, PROJ, ATT, CLS = 256, 300, 200, 200, 3
B, NCORES = 128, 8
NIT = B // NCORES            # items per core
NPAIR = NIT // 2
VOCAB = 50000

D_SL = [(0, 128), (128, 256), (256, 300)]          # EMB k-tiles
A_SL = [(0, 128), (128, 200)]                      # ATT/PROJ tiles
WC_K = [(0, 128), (128, 200), (200, 328), (328, 400)]
V_CH = [(0, 128), (128, 256), (256, 384), (384, 400)]  # P transpose chunks

_CACHED_NC = None


def _build_nc():
    nc = bacc.Bacc("TRN2", target_bir_lowering=False, debug=False)

    dram = {}
    def din(name, shape, dt):
        dram[name] = nc.dram_tensor(name, shape, dt, kind="ExternalInput")
        return dram[name]

    din("idx1", [128, 2 * NIT], I32)
    din("idx2", [128, 2 * NIT], I32)
    din("xi1", [NIT, L], I32)
    din("xi2", [NIT, L], I32)
    din("emb", [VOCAB, EMB], BF16)
    din("wi", [EMB, ATT], BF16)
    din("wp", [2 * EMB, PROJ], BF16)
    din("wa", [PROJ, ATT], BF16)
    din("wc", [2 * PROJ, 2 * PROJ], BF16)
    din("wg", [4 * PROJ, CLS], BF16)
    din("bi", [ATT, 1], F32)
    din("bp", [PROJ, 1], F32)
    din("ba_row", [1, ATT], BF16)
    din("bc_row", [1, 2 * PROJ], BF16)
    din("bg_row", [1, CLS], BF16)
    din("dmask", [L, L], BF16)
    din("bdist", [128, 1], F32)
    out_d = nc.dram_tensor("out", [CLS, NIT], F32, kind="ExternalOutput")

    with tile.TileContext(nc) as tc:
        _emit(nc, tc, dram, out_d)
    nc.compile()
    return nc


def _emit(nc, tc, dram, out_d):
    from contextlib import ExitStack
    ctx = ExitStack()
    with ctx:
        C = ctx.enter_context(tc.tile_pool(name="consts", bufs=1))
        PS = ctx.enter_context(tc.tile_pool(name="ps", bufs=8, space="PSUM"))
        W = ctx.enter_context(tc.tile_pool(name="work", bufs=3))

        def ps_tile(shape, dt=F32):
            return PS.tile(shape, dt, tag="ps", name="ps")

        def wtile(tag, shape=(128, 512), dt=BF16, bufs=3):
            return W.tile(list(shape), dt, tag=tag, name=tag, bufs=bufs)

        # ---------------- small input DMAs first ----------------
        idx_sb = {}
        for s, name in ((0, "idx1"), (1, "idx2")):
            t = C.tile([128, 2 * NIT], I32, tag=name, name=name)
            nc.sync.dma_start(out=t[:], in_=dram[name].ap())
            idx_sb[s] = t
        xi_sb = {}
        for s, name in ((0, "xi1"), (1, "xi2")):
            t = C.tile([NIT, L], I32, tag=name, name=name)
            nc.sync.dma_start(out=t[:], in_=dram[name].ap())
            xi_sb[s] = t

        dmask_d = dram["dmask"].ap()
        dmask_sb = []
        for (t0, t1) in [(0, 128), (128, 256)]:
            t = C.tile([128, 256], BF16, tag=f"dmask_{t0}", name=f"dmask_{t0}")
            nc.sync.dma_start(out=t[:t1 - t0, :], in_=dmask_d[t0:t1, :])
            dmask_sb.append(t)
        bdist = C.tile([128, 1], F32, tag="bdist", name="bdist")
        nc.sync.dma_start(out=bdist[:], in_=dram["bdist"].ap())

        # ---------------- constants ----------------
        ident_f = C.tile([128, 128], F32)
        make_identity(nc, ident_f[:])
        ident = C.tile([128, 128], BF16)
        nc.vector.tensor_copy(ident[:], ident_f[:])
        ones_bf = C.tile([1, 512], BF16)
        nc.vector.memset(ones_bf[:], 1.0)
        iota_i = C.tile([NIT, L], I32)
        nc.gpsimd.iota(iota_i[:], pattern=[[1, L]], base=0, channel_multiplier=0)
        iota16 = C.tile([NIT, L], F32)
        nc.vector.tensor_copy(iota16[:], iota_i[:])

        # ---------------- embedding gathers + squares (gpsimd) ----------------
        e_n = {}
        invsq = C.tile([128, 8 * NPAIR], F32)
        inv = C.tile([128, 8 * NPAIR], F32)

        def gcol(p, s, ti, h):
            return 8 * p + 4 * s + 2 * ti + h

        for p in range(NPAIR):
            for s in range(2):
                for ti in range(2):
                    t = C.tile([128, 600], BF16, tag=f"e_{s}_{ti}_{p}",
                               name=f"e_{s}_{ti}_{p}")
                    e_n[(s, ti, p)] = t
                    for h in range(2):
                        g = 2 * (2 * p + h) + ti
                        nc.gpsimd.indirect_dma_start(
                            out=t[:, h * 300:(h + 1) * 300], out_offset=None,
                            in_=dram["emb"].ap(),
                            in_offset=bass.IndirectOffsetOnAxis(
                                ap=idx_sb[s][:, g:g + 1], axis=0))

        # ---------------- masks ----------------
        m_bf = {}
        mcol_f = {}
        mcol_b = {}
        for s in range(2):
            xf = W.tile([NIT, L], F32, tag="xf", name="xf", bufs=1)
            nc.vector.tensor_copy(xf[:], xi_sb[s][:])
            nz = W.tile([NIT, L], F32, tag="nz", name="nz", bufs=1)
            nc.vector.tensor_scalar(nz[:], xf[:], 0.0, None, op0=ALU.not_equal)
            sizes = C.tile([NIT, 1], F32, tag=f"sizes{s}", name=f"sizes{s}")
            nc.vector.tensor_reduce(sizes[:], nz[:], axis=AX.X, op=ALU.add)
            mb = C.tile([NIT, L], BF16, tag=f"mbf{s}", name=f"mbf{s}")
            nc.vector.tensor_scalar(mb[:], iota16[:], sizes[:, :1], None,
                                    op0=ALU.is_lt)
            m_bf[s] = mb
            cf, cb = [], []
            for ti in range(2):
                pt = ps_tile([128, NIT], BF16)
                nc.tensor.transpose(pt[:, :NIT], mb[:NIT, ti * 128:(ti + 1) * 128],
                                    ident[:NIT, :NIT])
                f = C.tile([128, NIT], F32, tag=f"mcf{s}_{ti}", name=f"mcf{s}_{ti}")
                nc.vector.tensor_copy(f[:], pt[:, :NIT])
                bt = C.tile([128, NIT], BF16, tag=f"mcb{s}_{ti}", name=f"mcb{s}_{ti}")
                nc.scalar.copy(bt[:], pt[:, :NIT])
                cf.append(f)
                cb.append(bt)
            mcol_f[s] = cf
            mcol_b[s] = cb

        # ---------------- weights ----------------
        def load(name, r0, r1, dt=BF16):
            src = dram[name].ap()
            w = src.shape[1]
            t = C.tile([128, w], dt, tag=f"{name}_{r0}", name=f"{name}_{r0}")
            nc.sync.dma_start(out=t[:r1 - r0, :], in_=src[r0:r1, :])
            return t

        wi_k = [load("wi", d0, d1) for (d0, d1) in D_SL]
        wp_k = [load("wp", d0, d1) for (d0, d1) in D_SL] + \
               [load("wp", 300 + d0, 300 + d1) for (d0, d1) in D_SL]
        wa_k = [load("wa", a0, a1) for (a0, a1) in A_SL]
        wc_k = [load("wc", k0, k1) for (k0, k1) in WC_K]
        wg_k = [load("wg", s * 400 + v0, s * 400 + v1)
                for s in range(2) for (v0, v1) in V_CH]
        bi_sl = [load("bi", a0, a1, F32) for (a0, a1) in A_SL]
        bp_sl = [load("bp", p0, p1, F32) for (p0, p1) in A_SL]
        ba_row = load("ba_row", 0, 1)
        bc_row = load("bc_row", 0, 1)
        bg_row = load("bg_row", 0, 1)

        # bias2d pair tiles [x-blk, 512] bf16 = bdist * (dist>=10), duplicated
        bias2d = []
        for xb in range(2):
            b2 = C.tile([128, 512], BF16, tag=f"bias2d_{xb}", name=f"bias2d_{xb}")
            for h in range(2):
                nc.vector.tensor_scalar_mul(
                    b2[:, h * 256:(h + 1) * 256], dmask_sb[xb][:], bdist[:, :1])
            bias2d.append(b2)

        P_f = [C.tile([NIT, 400], F32, tag=f"P{s}", name=f"P{s}") for s in range(2)]

        # ---------------- pipeline stages ----------------
        state = {}

        def rden_pre(denst, prefix):
            """denst [128,4] f32 cols (h,blk) -> rrow [1,512] bf16 of
            reciprocal denominators (via PE column->row transposes)."""
            rden = wtile(f"{prefix}_rd", (128, 4), F32, bufs=4)
            nc.vector.reciprocal(rden[:], denst[:])
            rdbf = wtile(f"{prefix}_rdb", (128, 4), BF16, bufs=4)
            nc.vector.tensor_copy(rdbf[:], rden[:])
            rowps = ps_tile([1, 512], BF16)
            for h in range(2):
                for blk in range(2):
                    c = 2 * h + blk
                    nc.tensor.transpose(
                        rowps[:1, h * 256 + blk * 128: h * 256 + (blk + 1) * 128],
                        rdbf[:, c:c + 1], ident[:])
            rrow = wtile(f"{prefix}_rr", (1, 512), BF16, bufs=4)
            nc.scalar.copy(rrow[:], rowps[:1, :])
            return rrow

        def rden_bcast(rrow, prefix):
            bps = ps_tile([128, 512])
            nc.tensor.matmul(bps[:, :], lhsT=ones_bf[:1, :128], rhs=rrow[:1, :],
                             start=True, stop=True)
            rb = wtile(f"{prefix}_rb", (128, 512), BF16, bufs=2)
            nc.vector.tensor_copy(rb[:], bps[:, :])
            return rb

        def stage0(p):
            """norms (squares + Newton rsqrt + scale), eT, fT for both sides."""
            st = state.setdefault(p, {})
            for s in range(2):
                for ti in range(2):
                    for h in range(2):
                        g = gcol(p, s, ti, h)
                        scr = W.tile([128, 300], BF16, tag="sqscr", name="sqscr",
                                     bufs=4)
                        src_ap = e_n[(s, ti, p)][:, h * 300:(h + 1) * 300]
                        if (ti + h) % 2 == 0:
                            nc.scalar.activation(scr[:], src_ap, AF.Square,
                                                 accum_out=invsq[:, g:g + 1])
                        else:
                            nc.vector.scalar_tensor_tensor(
                                scr[:], src_ap, 1.0, src_ap,
                                op0=ALU.mult, op1=ALU.mult,
                                accum_out=invsq[:, g:g + 1])
            # Newton rsqrt on [128, 8]: magic seed + 2 iterations
            c0, c1 = 8 * p, 8 * p + 8
            x = invsq[:, c0:c1]
            it_ = wtile("nwt_i", (128, 8), I32, bufs=2)
            nc.vector.tensor_scalar(it_[:], x.bitcast(I32), 1, None,
                                    op0=ALU.arith_shift_right)
            nc.vector.tensor_scalar(it_[:], it_[:], -1, 0x5F3759DF,
                                    op0=ALU.mult, op1=ALU.add)
            y = it_[:].bitcast(F32)
            t1 = wtile("nwt_t", (128, 8), F32, bufs=2)
            for _ in range(2):
                nc.vector.tensor_mul(t1[:], y, y)
                nc.vector.tensor_mul(t1[:], t1[:], x)
                nc.vector.tensor_scalar(t1[:], t1[:], -0.5, 1.5,
                                        op0=ALU.mult, op1=ALU.add)
                nc.vector.tensor_mul(y, y, t1[:])
            nc.vector.tensor_copy(inv[:, c0:c1], y)
            for s in range(2):
                for ti in range(2):
                    for h in range(2):
                        g = gcol(p, s, ti, h)
                        t = e_n[(s, ti, p)]
                        nc.vector.tensor_scalar_mul(
                            t[:, h * 300:(h + 1) * 300],
                            t[:, h * 300:(h + 1) * 300], inv[:, g:g + 1])
            for s in range(2):
                eT = []
                for di, (d0, d1) in enumerate(D_SL):
                    dsz = d1 - d0
                    tps = ps_tile([128, 512], BF16)
                    for h in range(2):
                        for ti in range(2):
                            nc.tensor.transpose(
                                tps[:dsz, h * 256 + ti * 128:
                                    h * 256 + (ti + 1) * 128],
                                e_n[(s, ti, p)][:, h * 300 + d0: h * 300 + d1],
                                ident[:])
                    t = wtile(f"eT{s}{di}", bufs=3)
                    nc.vector.tensor_copy(t[:dsz, :], tps[:dsz, :])
                    eT.append(t)
                st[f"eT{s}"] = eT
            for s in range(2):
                fT = []
                for ai, (a0, a1) in enumerate(A_SL):
                    asz = a1 - a0
                    ps = ps_tile([128, 512])
                    for k in range(3):
                        ksz = D_SL[k][1] - D_SL[k][0]
                        nc.tensor.matmul(ps[:asz, :],
                                         lhsT=wi_k[k][:ksz, a0:a1],
                                         rhs=st[f"eT{s}"][k][:ksz, :],
                                         start=(k == 0), stop=(k == 2))
                    t = wtile(f"fT{s}{ai}", bufs=2)
                    nc.scalar.activation(t[:asz, :], ps[:asz, :], AF.Relu,
                                         bias=bi_sl[ai][:asz, :1])
                    fT.append(t)
                st[f"fT{s}"] = fT

        def stage1a(p):
            """att (+bias via identity matmul) and exps for both sides."""
            st = state[p]
            for s in range(2):
                fT = st[f"fT{s}"]
                denst = wtile(f"iden{s}", (128, 4), F32, bufs=3)
                E = []
                att_ps = []
                for xb in range(2):
                    ps = ps_tile([128, 512])
                    for h in range(2):
                        for ai, (a0, a1) in enumerate(A_SL):
                            asz = a1 - a0
                            nc.tensor.matmul(
                                ps[:, h * 256:(h + 1) * 256],
                                lhsT=fT[ai][:asz, h * 256 + xb * 128:
                                            h * 256 + (xb + 1) * 128],
                                rhs=fT[ai][:asz, h * 256:(h + 1) * 256],
                                start=(ai == 0), stop=False)
                        nc.tensor.matmul(
                            ps[:, h * 256:(h + 1) * 256],
                            lhsT=ident[:, :], rhs=bias2d[xb][:, h * 256:(h + 1) * 256],
                            start=False, stop=True)
                    att_ps.append(ps)
                for xb in range(2):
                    et = wtile(f"E{s}{xb}", bufs=2)
                    for h in range(2):
                        nc.scalar.activation(
                            et[:, h * 256:(h + 1) * 256],
                            att_ps[xb][:, h * 256:(h + 1) * 256], AF.Exp,
                            accum_out=denst[:, 2 * h + xb: 2 * h + xb + 1])
                    E.append(et)
                st[f"E{s}"] = E
                st[f"denI{s}"] = denst

        def stage1b(p):
            """rden prep, then per side: xp matmuls, broadcast, drains."""
            st = state[p]
            rrows = [rden_pre(st[f"denI{s}"], f"i{s}") for s in range(2)]
            for s in range(2):
                E = st[f"E{s}"]
                xp_ps = []
                for di, (d0, d1) in enumerate(D_SL):
                    dsz = d1 - d0
                    ps = ps_tile([128, 512])
                    for h in range(2):
                        for ti in range(2):
                            nc.tensor.matmul(
                                ps[:dsz, h * 256:(h + 1) * 256],
                                lhsT=e_n[(s, ti, p)][:, h * 300 + d0: h * 300 + d1],
                                rhs=E[ti][:, h * 256:(h + 1) * 256],
                                start=(ti == 0), stop=(ti == 1))
                    xp_ps.append(ps)
                rb = rden_bcast(rrows[s], f"i{s}")
                xpT = []
                for di, (d0, d1) in enumerate(D_SL):
                    dsz = d1 - d0
                    t = wtile(f"xp{s}{di}", bufs=2)
                    nc.vector.tensor_mul(t[:dsz, :], xp_ps[di][:dsz, :],
                                         rb[:dsz, :])
                    xpT.append(t)
                st[f"xp{s}"] = xpT

        def stage2(p):
            """pT, pRow, aT for both sides."""
            st = state[p]
            for s in range(2):
                hT = st[f"eT{s}"] + st[f"xp{s}"]
                pT = []
                for pi, (p0, p1) in enumerate(A_SL):
                    psz = p1 - p0
                    ps = ps_tile([128, 512])
                    for k in range(6):
                        ksz = D_SL[k % 3][1] - D_SL[k % 3][0]
                        nc.tensor.matmul(ps[:psz, :], lhsT=wp_k[k][:ksz, p0:p1],
                                         rhs=hT[k][:ksz, :], start=(k == 0),
                                         stop=(k == 5))
                    t = wtile(f"pT{s}{pi}", bufs=3)
                    nc.scalar.activation(t[:psz, :], ps[:psz, :], AF.Identity,
                                         bias=bp_sl[pi][:psz, :1])
                    pT.append(t)
                st[f"pT{s}"] = pT
            for s in range(2):
                pT = st[f"pT{s}"]
                pRow = []
                for ti in range(2):
                    tps = ps_tile([128, 400], BF16)
                    for h in range(2):
                        for pi, (p0, p1) in enumerate(A_SL):
                            psz = p1 - p0
                            nc.tensor.transpose(
                                tps[:, h * 200 + p0: h * 200 + p1],
                                pT[pi][:psz, h * 256 + ti * 128:
                                       h * 256 + (ti + 1) * 128],
                                ident[:psz, :psz])
                    t = wtile(f"pR{s}{ti}", (128, 400), bufs=3)
                    nc.scalar.copy(t[:], tps[:, :])
                    pRow.append(t)
                st[f"pR{s}"] = pRow
            maskrow = {}
            for s in range(2):
                rps = ps_tile([1, 512], BF16)
                for h in range(2):
                    it = 2 * p + h
                    for ti in range(2):
                        nc.tensor.transpose(
                            rps[:1, h * 256 + ti * 128: h * 256 + (ti + 1) * 128],
                            mcol_b[s][ti][:, it:it + 1], ident[:])
                mrow = wtile(f"mrow{s}", (1, 512), BF16, bufs=2)
                nc.vector.tensor_copy(mrow[:], rps[:1, :])
                bps = ps_tile([128, 512])
                nc.tensor.matmul(bps[:, :], lhsT=ones_bf[:1, :128],
                                 rhs=mrow[:1, :], start=True, stop=True)
                mr = wtile(f"mrowB{s}", (128, 512), BF16, bufs=2)
                nc.vector.tensor_copy(mr[:], bps[:, :])
                maskrow[s] = mr
            for s in range(2):
                pT = st[f"pT{s}"]
                aT = []
                for ai, (a0, a1) in enumerate(A_SL):
                    asz = a1 - a0
                    ps = ps_tile([128, 512])
                    for ki, (k0, k1) in enumerate(A_SL):
                        ksz = k1 - k0
                        nc.tensor.matmul(ps[:asz, :], lhsT=wa_k[ki][:ksz, a0:a1],
                                         rhs=pT[ki][:ksz, :], start=(ki == 0),
                                         stop=False)
                    nc.tensor.matmul(ps[:asz, :], lhsT=ba_row[:1, a0:a1],
                                     rhs=ones_bf[:1, :], start=False, stop=True)
                    t = wtile(f"aT{s}{ai}", bufs=2)
                    nc.vector.scalar_tensor_tensor(
                        t[:asz, :], ps[:asz, :], 0.0, maskrow[s][:asz, :],
                        op0=ALU.max, op1=ALU.mult)
                    aT.append(t)
                st[f"aT{s}"] = aT

        def stage3a(p):
            """sim & simT matmuls and exps with mask scale."""
            st = state[p]
            a1T, a2T = st["aT0"], st["aT1"]
            den2 = wtile("den2", (128, 4), F32, bufs=3)
            den1 = wtile("den1", (128, 4), F32, bufs=3)
            E2, E1 = [], []
            sim_ps, simT_ps = [], []
            for xb in range(2):
                ps = ps_tile([128, 512])
                for h in range(2):
                    for ai, (a0, a1) in enumerate(A_SL):
                        asz = a1 - a0
                        nc.tensor.matmul(
                            ps[:, h * 256:(h + 1) * 256],
                            lhsT=a1T[ai][:asz, h * 256 + xb * 128:
                                         h * 256 + (xb + 1) * 128],
                            rhs=a2T[ai][:asz, h * 256:(h + 1) * 256],
                            start=(ai == 0), stop=(ai == 1))
                sim_ps.append(ps)
            for xb in range(2):
                et = wtile(f"E2_{xb}", bufs=2)
                for h in range(2):
                    it = 2 * p + h
                    nc.scalar.activation(
                        et[:, h * 256:(h + 1) * 256],
                        sim_ps[xb][:, h * 256:(h + 1) * 256], AF.Exp,
                        scale=mcol_f[0][xb][:, it:it + 1],
                        accum_out=den2[:, 2 * h + xb: 2 * h + xb + 1])
                E2.append(et)
            for yb in range(2):
                ps = ps_tile([128, 512])
                for h in range(2):
                    for ai, (a0, a1) in enumerate(A_SL):
                        asz = a1 - a0
                        nc.tensor.matmul(
                            ps[:, h * 256:(h + 1) * 256],
                            lhsT=a2T[ai][:asz, h * 256 + yb * 128:
                                         h * 256 + (yb + 1) * 128],
                            rhs=a1T[ai][:asz, h * 256:(h + 1) * 256],
                            start=(ai == 0), stop=(ai == 1))
                simT_ps.append(ps)
            for yb in range(2):
                et = wtile(f"E1_{yb}", bufs=2)
                for h in range(2):
                    it = 2 * p + h
                    nc.scalar.activation(
                        et[:, h * 256:(h + 1) * 256],
                        simT_ps[yb][:, h * 256:(h + 1) * 256], AF.Exp,
                        scale=mcol_f[1][yb][:, it:it + 1],
                        accum_out=den1[:, 2 * h + yb: 2 * h + yb + 1])
                E1.append(et)
            st["E2"], st["E1"] = E2, E1
            st["den2"], st["den1"] = den2, den1

        def stage3b(p):
            """betaT / alphaT matmuls with drain-time normalization."""
            st = state[p]
            rr2 = rden_pre(st["den2"], "x2")
            betaT, alphaT = [], []
            beta_ps = []
            for pi, (p0, p1) in enumerate(A_SL):
                psz = p1 - p0
                ps = ps_tile([128, 512])
                for h in range(2):
                    for ti in range(2):
                        nc.tensor.matmul(
                            ps[:psz, h * 256:(h + 1) * 256],
                            lhsT=st["pR1"][ti][:, h * 200 + p0: h * 200 + p1],
                            rhs=st["E1"][ti][:, h * 256:(h + 1) * 256],
                            start=(ti == 0), stop=(ti == 1))
                beta_ps.append(ps)
            b2 = rden_bcast(rr2, "x2")
            for pi, (p0, p1) in enumerate(A_SL):
                psz = p1 - p0
                t = wtile(f"bT{pi}", bufs=2)
                nc.vector.tensor_mul(t[:psz, :], beta_ps[pi][:psz, :],
                                     b2[:psz, :])
                betaT.append(t)
            rr1 = rden_pre(st["den1"], "x1")
            alpha_ps = []
            for pi, (p0, p1) in enumerate(A_SL):
                psz = p1 - p0
                ps = ps_tile([128, 512])
                for h in range(2):
                    for xb in range(2):
                        nc.tensor.matmul(
                            ps[:psz, h * 256:(h + 1) * 256],
                            lhsT=st["pR0"][xb][:, h * 200 + p0: h * 200 + p1],
                            rhs=st["E2"][xb][:, h * 256:(h + 1) * 256],
                            start=(xb == 0), stop=(xb == 1))
                alpha_ps.append(ps)
            b1 = rden_bcast(rr1, "x1")
            for pi, (p0, p1) in enumerate(A_SL):
                psz = p1 - p0
                t = wtile(f"alT{pi}", bufs=2)
                nc.vector.tensor_mul(t[:psz, :], alpha_ps[pi][:psz, :],
                                     b1[:psz, :])
                alphaT.append(t)
            st["betaT"], st["alphaT"] = betaT, alphaT

        def stage5(p):
            """compare + bias + relu + PE pooling + row stash."""
            st = state[p]
            for s, pTt, oT in ((0, st["pT0"], st["betaT"]),
                               (1, st["pT1"], st["alphaT"])):
                kt = pTt + oT
                for h in range(2):
                    it = 2 * p + h
                    vrs = []
                    for ti in range(2):
                        cps = ps_tile([128, 400])
                        for k in range(4):
                            ksz = WC_K[k][1] - WC_K[k][0]
                            nc.tensor.matmul(
                                cps[:, :],
                                lhsT=kt[k][:ksz, h * 256 + ti * 128:
                                           h * 256 + (ti + 1) * 128],
                                rhs=wc_k[k][:ksz, :400],
                                start=(k == 0), stop=False)
                        nc.tensor.matmul(cps[:, :], lhsT=ones_bf[:1, :128],
                                         rhs=bc_row[:1, :400],
                                         start=False, stop=True)
                        vr = wtile("vr", (128, 400), BF16, bufs=3)
                        if ti == 0:
                            nc.vector.tensor_scalar(vr[:], cps[:, :], 0.0, None,
                                                    op0=ALU.max)
                        else:
                            nc.scalar.activation(vr[:], cps[:, :], AF.Relu)
                        vrs.append(vr)
                    pps = ps_tile([1, 400])
                    for ti in range(2):
                        nc.tensor.matmul(pps[:1, :],
                                         lhsT=mcol_b[s][ti][:, it:it + 1],
                                         rhs=vrs[ti][:, :],
                                         start=(ti == 0), stop=(ti == 1))
                    prow = wtile("prow", (1, 400), F32, bufs=3)
                    if s == 0:
                        nc.scalar.copy(prow[:], pps[:1, :])
                    else:
                        nc.vector.tensor_copy(prow[:], pps[:1, :])
                    nc.sync.dma_start(out=P_f[s][it:it + 1, :], in_=prow[:1, :])
            del state[p]

        stages = [stage0, stage1a, stage1b, stage2, stage3a, stage3b, stage5]
        NST = len(stages)
        for t in range(NPAIR + NST - 1):
            for k in reversed(range(NST)):
                p = t - k
                if 0 <= p < NPAIR:
                    stages[k](p)

        # ---------------- aggregate ----------------
        PT_sb = []
        for s in range(2):
            pb = C.tile([NIT, 400], BF16, tag=f"Pb{s}", name=f"Pb{s}")
            nc.vector.tensor_copy(pb[:], P_f[s][:])
            for c, (c0, c1) in enumerate(V_CH):
                csz = c1 - c0
                tps = ps_tile([128, NIT], BF16)
                nc.tensor.transpose(tps[:csz, :NIT], pb[:NIT, c0:c1],
                                    ident[:NIT, :NIT])
                t = C.tile([128, NIT], BF16, tag=f"PT{s}_{c}", name=f"PT{s}_{c}")
                nc.scalar.copy(t[:csz, :], tps[:csz, :])
                PT_sb.append(t)
        aps = ps_tile([CLS, NIT])
        for k in range(8):
            ksz = V_CH[k % 4][1] - V_CH[k % 4][0]
            nc.tensor.matmul(aps[:, :], lhsT=wg_k[k][:ksz, :CLS],
                             rhs=PT_sb[k][:ksz, :], start=(k == 0), stop=False)
        nc.tensor.matmul(aps[:, :], lhsT=bg_row[:1, :CLS],
                         rhs=ones_bf[:1, :NIT], start=False, stop=True)
        out_sb = C.tile([CLS, NIT], F32)
        nc.scalar.copy(out_sb[:], aps[:, :])
        nc.sync.dma_start(out=out_d.ap(), in_=out_sb[:])


def _get_nc():
    global _CACHED_NC
    if _CACHED_NC is None:
        _CACHED_NC = _build_nc()
    return _CACHED_NC


def make_in_maps(inputs):
    x1 = np.asarray(inputs["x1"])
    x2 = np.asarray(inputs["x2"])
    f32 = lambda k: np.ascontiguousarray(np.asarray(inputs[k], dtype=np.float32))
    bf = lambda a: np.ascontiguousarray(np.asarray(a, dtype=np.float32)).astype(BF_NP)
    ii, jj = np.meshgrid(np.arange(L), np.arange(L), indexing="ij")
    dmask = (np.abs(ii - jj) >= 10).astype(np.float32)
    bdist = np.full((128, 1), np.asarray(inputs["b_dist"], np.float32).reshape(-1)[0],
                    np.float32)

    shared = {
        "emb": bf(inputs["emb"]),
        "wi": bf(inputs["Wi"]), "wp": bf(inputs["Wp"]), "wa": bf(inputs["Wa"]),
        "wc": bf(inputs["Wc"]), "wg": bf(inputs["Wg"]),
        "bi": f32("bi").reshape(-1, 1), "bp": f32("bp").reshape(-1, 1),
        "ba_row": bf(np.asarray(inputs["ba"]).reshape(1, -1)),
        "bc_row": bf(np.asarray(inputs["bc"]).reshape(1, -1)),
        "bg_row": bf(np.asarray(inputs["bg"]).reshape(1, -1)),
        "dmask": dmask.astype(BF_NP), "bdist": bdist,
    }
    in_maps = []
    for c in range(NCORES):
        sl = slice(c * NIT, (c + 1) * NIT)
        x1s = np.ascontiguousarray(x1[sl]).astype(np.int32)
        x2s = np.ascontiguousarray(x2[sl]).astype(np.int32)
        m = dict(shared)
        m["idx1"] = np.ascontiguousarray(x1s.reshape(-1).reshape(2 * NIT, 128).T)
        m["idx2"] = np.ascontiguousarray(x2s.reshape(-1).reshape(2 * NIT, 128).T)
        m["xi1"] = x1s
        m["xi2"] = x2s
        in_maps.append(m)
    return in_maps


def kernel(**inputs):
    nc = _get_nc()
    in_maps = make_in_maps(inputs)
    res = run_bass_kernel_spmd(nc, in_maps, core_ids=list(range(NCORES)))
    out = np.concatenate([r["out"].T for r in res.results], axis=0)
    return np.ascontiguousarray(out, dtype=np.float32)
